# revision 3
# baseline (speedup 1.0000x reference)
"""ContrastiveLoss kernel v2 for 8 Trainium2 NeuronCores (Bass/Tile, SPMD).

Problem (B=8192, D=512, fp32):
  n = ||x1||_row;  sim12 = rowdot(x1, x2) / (n1*n2);  p = exp(sim12)
  G = (x1 @ x1.T) / (n n^T);  E = exp(G)
  neg_j = sum_k E[j,k] - E[j, (j-1) % B]
  loss = mean_j( log(p_j + neg_j) - sim12_j )

v2 restructure vs baseline (157us):
  - HAM warmup MM stream at t=0 so the whole gram runs at 2.4GHz
  - ACT table preload at t=0
  - ONE psum pool of [128, 2048] f32 tiles (4 banks x 2 bufs = all 8
    banks). Norm/product column-sum matmuls bootstrap through the same
    ring, interleaved so no ring slot ever waits on late work:
      blockp, np0, g00..g05, np1, g06..g07, g10..g15, np2, g16..g17,
      g20..g21, prodp, prodp2, g22..g25, np3?? (np3 before g25) ... fin
  - gram exp ACTs 2048-wide (halves per-instruction 352-cyc overhead)
  - yf8 production: y16 = x*inv16 (TT bf16 2x) then f8 via tensor_scalar
    (2x) -- the baseline's fused TT with f8 dst ran at 1x
  - finals in transposed [128, 8] layout via a DRAM bounce (tail ~4us
    instead of baseline's ~24us)
  - rowsums: ScalarE accum_out vs DVE tensor_reduce split, tunable
"""

import sys
import types

import ml_dtypes
import numpy as np

BF16 = ml_dtypes.bfloat16

B = 8192
D = 512
NCORES = 8
BLK = B // NCORES  # 1024
KT = D // 128  # 4 k-tiles
BW = BLK + 1  # block width incl wrap column
NPAIR = 4  # chunk pairs
PW = 2048  # pair width

# rowsum split: tile idx with (idx % RS_MOD) < RS_SC -> ScalarE accum_out,
# else DVE tensor_reduce on the bf16 esc tile
RS_SC = 4
RS_MOD = 4
# offload squares of k in GP_SQ_K (pairs >= 1) to GPSIMD
GP_SQ_K = ()


def _install_ntff_shim():
    if "antenv.axon_hooks" in sys.modules:
        return
    mod = types.ModuleType("antenv.axon_hooks")
    mod._hook = None

    def set_axon_ntff_profile_hook(h):
        mod._hook = h

    def get_axon_ntff_profile_hook():
        return mod._hook

    mod.set_axon_ntff_profile_hook = set_axon_ntff_profile_hook
    mod.get_axon_ntff_profile_hook = get_axon_ntff_profile_hook
    sys.modules["antenv.axon_hooks"] = mod
    try:
        import antenv

        antenv.axon_hooks = mod
    except ImportError:
        pass
    try:
        from trn_agent_boot.trn_boot import _ntff_profile_via_ctypes

        hook = _ntff_profile_via_ctypes("/opt/axon/libaxon_pjrt.so")
        if hook is not None:
            set_axon_ntff_profile_hook(hook)
    except Exception:
        pass


def build_program():
    _install_ntff_shim()
    import concourse.bass as bass
    import concourse.tile as tile
    from concourse import mybir

    f32 = mybir.dt.float32
    bf16 = mybir.dt.bfloat16
    f8 = mybir.dt.float8e4
    AF = mybir.ActivationFunctionType
    ALU = mybir.AluOpType
    AX = mybir.AxisListType

    nc = bass.Bass("TRN2", target_bir_lowering=False, debug=False, num_devices=NCORES)

    x1t = nc.declare_dram_parameter("x1t", [D, B], bf16, isOutput=False)
    x1tb = nc.declare_dram_parameter("x1tb", [D, BW], bf16, isOutput=False)
    x2t = nc.declare_dram_parameter("x2t", [D, BLK], bf16, isOutput=False)
    out = nc.declare_dram_parameter("out", [1, 1], f32, isOutput=True)
    prod_bounce = nc.dram_tensor("prod_bounce", [1, 3072], f32)

    with tile.TileContext(nc) as tc:
        with (
            tc.tile_pool(name="const", bufs=1) as constp,
            tc.tile_pool(name="big", bufs=1) as bigp,
            tc.tile_pool(name="xbp", bufs=12) as xbp,
            tc.tile_pool(name="sqp", bufs=3) as sqp,
            tc.tile_pool(name="sqwp", bufs=8) as sqwp,
            tc.tile_pool(name="y16p", bufs=2) as y16p,
            tc.tile_pool(name="invp", bufs=2) as invp,
            tc.tile_pool(name="lncp", bufs=2) as lncp,
            tc.tile_pool(name="escp", bufs=3) as escp,
            tc.tile_pool(name="finp", bufs=1) as finp,
            tc.tile_pool(name="gp", bufs=2, space=bass.MemorySpace.PSUM) as gpp,
        ):
            # ------------- constants + engine warmers -------------
            ones = constp.tile([128, 128], bf16, tag="ones")
            nc.vector.memset(ones[:], 1.0)
            ones1 = ones[:, 0:1]
            ln16 = constp.tile([128, 1], f32, tag="ln16")
            nc.vector.memset(ln16[:], 2.772588722239781)
            negln16 = constp.tile([128, 1], f32, tag="negln16")
            nc.vector.memset(negln16[:], -2.772588722239781)
            actwarm = constp.tile([128, 1], f32, tag="actwarm")
            nc.scalar.activation(actwarm[:], ln16[:], AF.Ln)

            # ------------- input DMAs (issue order = priority) ----
            yb = [
                bigp.tile([128, BW], bf16, tag=f"yb{k}", name=f"yb{k}")
                for k in range(KT)
            ]
            for k in range(KT):
                nc.sync.dma_start(yb[k][:, :], x1tb[k * 128 : (k + 1) * 128, :])
            xb = {}

            def dma_pair(pc):
                for k in range(KT):
                    t_ = xbp.tile([128, PW], bf16, tag="xb", name=f"xb_p{pc}k{k}")
                    nc.sync.dma_start(
                        t_[:], x1t[k * 128 : (k + 1) * 128, pc * PW : (pc + 1) * PW]
                    )
                    xb[(pc, k)] = t_

            dma_pair(0)
            x2b = []
            for k in range(KT):
                t2 = bigp.tile([128, BLK], bf16, tag=f"x2b{k}", name=f"x2b{k}")
                nc.sync.dma_start(t2[:], x2t[k * 128 : (k + 1) * 128, :])
                x2b.append(t2)
            for pc in range(1, NPAIR):
                dma_pair(pc)

            # ------------- ring tile 0: warmup + block norms ------
            blockp = gpp.tile([128, PW], f32, tag="gp", name="blockp")
            for w in range(55):
                lo = 1024 + (w % 2) * 128
                nc.tensor.matmul(
                    blockp[:, lo : lo + 128], ones[:], ones[:], start=True, stop=True
                )
            sqb = [
                sqp.tile([128, BW], bf16, tag="sqb", name=f"sqb{k}") for k in range(KT)
            ]
            for k in range(KT):
                nc.vector.tensor_mul(sqb[k][:], yb[k][:, :], yb[k][:, :])
            for k in range(KT):
                st = k == 0
                sp = k == KT - 1
                for h in range(2):
                    nc.tensor.matmul(
                        blockp[:, h * 512 : (h + 1) * 512],
                        ones[:],
                        sqb[k][:, h * 512 : (h + 1) * 512],
                        start=st,
                        stop=sp,
                    )
                nc.tensor.matmul(
                    blockp[:, 1664:1665],
                    ones[:],
                    sqb[k][:, 1024:1025],
                    start=st,
                    stop=sp,
                )
            lnb = lncp.tile([128, PW], f32, tag="lnc", name="lnb")
            nc.scalar.activation(lnb[:, 0:1024], blockp[:, 0:1024], AF.Ln)
            nc.scalar.activation(lnb[:, 1024:1025], blockp[:, 1664:1665], AF.Ln)
            invb = constp.tile([128, BW], bf16, tag="invb")
            nc.scalar.activation(
                invb[:, 0:1024], lnb[:, 0:1024], AF.Exp, scale=-0.5, bias=ln16[:]
            )
            nc.scalar.activation(
                invb[:, 1024:1025], lnb[:, 1024:1025], AF.Exp, scale=-0.5, bias=ln16[:]
            )
            yb16 = [
                bigp.tile([128, BW], bf16, tag=f"yb16{k}", name=f"yb16{k}")
                for k in range(KT)
            ]
            ybf8 = [
                bigp.tile([128, 2, 1040], f8, tag=f"ybf8{t}", name=f"ybf8{t}")
                for t in range(2)
            ]

            def emit_block_tail():
                for k in range(KT):
                    nc.vector.tensor_mul(yb16[k][:], yb[k][:, :], invb[:])
                for k in range(KT):
                    nc.vector.tensor_scalar_mul(
                        ybf8[k // 2][:, k % 2, 0:BW], yb16[k][:], 1.0
                    )

            # ------------- normalize pipeline per pair ------------
            yf8 = [
                bigp.tile([128, 2, B], f8, tag=f"yf8{t}", name=f"yf8{t}")
                for t in range(2)
            ]

            invcs = {}

            def emit_norm_stats(pc):
                sqs = []
                for k in range(KT):
                    s_ = sqwp.tile([128, PW], bf16, tag="sqw", name=f"sq_p{pc}k{k}")
                    nc.vector.tensor_mul(s_[:], xb[(pc, k)][:], xb[(pc, k)][:])
                    sqs.append(s_)
                npt = gpp.tile([128, PW], f32, tag="gp", name=f"np{pc}")
                for k in range(KT):
                    st = k == 0
                    sp = k == KT - 1
                    for h in range(4):
                        nc.tensor.matmul(
                            npt[:, h * 512 : (h + 1) * 512],
                            ones[:],
                            sqs[k][:, h * 512 : (h + 1) * 512],
                            start=st,
                            stop=sp,
                        )
                lnc = lncp.tile([128, PW], f32, tag="lnc", name=f"lnc{pc}")
                nc.scalar.activation(lnc[:], npt[:], AF.Ln)
                invc = invp.tile([128, PW], bf16, tag="inv", name=f"inv{pc}")
                nc.scalar.activation(invc[:], lnc[:], AF.Exp, scale=-0.5, bias=ln16[:])
                invcs[pc] = invc

            def emit_ymul(pc):
                for k in range(KT):
                    nc.vector.tensor_mul(
                        yf8[k // 2][:, k % 2, pc * PW : (pc + 1) * PW],
                        xb[(pc, k)][:],
                        invcs[pc][:],
                    )

            # ------------- gram tile ------------------------------
            rs_acc = finp.tile([128, 32], f32, tag="rs_acc")
            rs_accD = finp.tile([128, 32], bf16, tag="rs_accD")
            nc.vector.memset(rs_acc[:], 0.0)
            nc.vector.memset(rs_accD[:], 0.0)

            def emit_gram(pp, r):
                gpt = gpp.tile([128, PW], f32, tag="gp", name=f"g_p{pp}r{r}")
                for t in range(2):
                    for qh in range(4):
                        col0 = pp * PW + qh * 512
                        nc.tensor.matmul(
                            gpt[:, qh * 512 : (qh + 1) * 512],
                            ybf8[t][:, :, r * 128 : (r + 1) * 128],
                            yf8[t][:, :, col0 : col0 + 512],
                            start=(t == 0),
                            stop=(t == 1),
                            perf_mode=mybir.MatmulPerfMode.DoubleRow,
                        )
                idx = r * 4 + pp
                esc = escp.tile([128, PW], bf16, tag="esc")
                if (idx % RS_MOD) < RS_SC:
                    nc.scalar.activation(
                        esc[:],
                        gpt[:],
                        AF.Exp,
                        scale=0.00390625,
                        accum_out=rs_acc[:, idx : idx + 1],
                    )
                else:
                    nc.scalar.activation(esc[:], gpt[:], AF.Exp, scale=0.00390625)
                    with nc.allow_low_precision(reason="rowsum partial in bf16, tol 2e-2"):
                        nc.vector.tensor_reduce(
                            rs_accD[:, idx : idx + 1], esc[:], axis=AX.X, op=ALU.add
                        )

            # ------------- products (emitted mid-stream) ----------
            def emit_products_muls():
                for k in range(KT):
                    zb = sqp.tile([128, BLK], bf16, tag="zz", name=f"zb{k}")
                    nc.vector.tensor_mul(
                        zb[:, 1:1024], yb16[k][:, 1:1024], yb16[k][:, 0:1023]
                    )
                    nc.vector.tensor_mul(
                        zb[:, 0:1], yb16[k][:, 0:1], yb16[k][:, 1024:1025]
                    )
                    self_zb.append(zb)
                for k in range(KT):
                    z2 = sqp.tile([128, BLK], bf16, tag="zz", name=f"z2{k}")
                    nc.vector.tensor_mul(z2[:], yb16[k][:, 0:1024], x2b[k][:])
                    self_z2.append(z2)
                for k in range(KT):
                    sq2 = sqp.tile([128, BLK], bf16, tag="zz", name=f"sq2{k}")
                    nc.vector.tensor_mul(sq2[:], x2b[k][:], x2b[k][:])
                    self_sq2.append(sq2)

            self_zb, self_z2, self_sq2 = [], [], []

            stage = finp.tile([1, 3072], f32, tag="stage")

            def emit_products_psum_a():
                prodp = gpp.tile([128, PW], f32, tag="gp", name="prodp")
                for k in range(KT):
                    st = k == 0
                    sp = k == KT - 1
                    for h in range(2):
                        nc.tensor.matmul(
                            prodp[0:1, h * 512 : (h + 1) * 512],
                            ones1,
                            self_zb[k][:, h * 512 : (h + 1) * 512],
                            start=st,
                            stop=sp,
                        )
                    for h in range(2):
                        nc.tensor.matmul(
                            prodp[0:1, 1024 + h * 512 : 1024 + (h + 1) * 512],
                            ones1,
                            self_z2[k][:, h * 512 : (h + 1) * 512],
                            start=st,
                            stop=sp,
                        )
                nc.scalar.activation(
                    stage[0:1, 0:1024], prodp[0:1, 0:1024], AF.Exp, scale=0.00390625
                )
                nc.scalar.activation(
                    stage[0:1, 1024:2048], prodp[0:1, 1024:2048], AF.Copy
                )

            def emit_products_psum_b():
                prodp2 = gpp.tile([128, PW], f32, tag="gp", name="prodp2")
                for k in range(KT):
                    st = k == 0
                    sp = k == KT - 1
                    for h in range(2):
                        nc.tensor.matmul(
                            prodp2[0:1, h * 512 : (h + 1) * 512],
                            ones1,
                            self_sq2[k][:, h * 512 : (h + 1) * 512],
                            start=st,
                            stop=sp,
                        )
                nc.scalar.activation(
                    stage[0:1, 2048:3072], prodp2[0:1, 0:1024], AF.Copy
                )

            def emit_products_finish():
                nc.sync.dma_start(prod_bounce[0:1, :], stage[0:1, :])
                prodT = finp.tile([128, 24], f32, tag="prodT")
                nc.sync.dma_start(
                    prodT[:, :].rearrange("p (v r) -> p v r", v=3),
                    prod_bounce[0:1, :].rearrange("a (v r p) -> (a p) v r", v=3, r=8),
                )
                ln2T = finp.tile([128, 8], f32, tag="ln2T")
                nc.scalar.activation(ln2T[:], prodT[:, 16:24], AF.Ln)
                inv2T = finp.tile([128, 8], f32, tag="inv2T")
                nc.scalar.activation(
                    inv2T[:], ln2T[:], AF.Exp, scale=-0.5, bias=negln16[:]
                )
                sim12T = finp.tile([128, 8], f32, tag="sim12T")
                nc.vector.tensor_mul(sim12T[:], prodT[:, 8:16], inv2T[:])
                posT = finp.tile([128, 8], f32, tag="posT")
                nc.scalar.activation(posT[:], sim12T[:], AF.Exp)
                s12sumT = finp.tile([128, 1], f32, tag="s12sumT")
                nc.vector.tensor_reduce(s12sumT[:], sim12T[:], axis=AX.X, op=ALU.add)
                return prodT[:, 0:8], posT, sim12T, s12sumT

            # ------------- emission schedule ----------------------
            emit_norm_stats(0)
            emit_norm_stats(1)
            emit_block_tail()
            emit_ymul(0)
            emit_gram(0, 0)
            emit_gram(0, 1)
            emit_norm_stats(2)
            emit_ymul(1)
            for r in range(2, 6):
                emit_gram(0, r)
            emit_norm_stats(3)
            emit_ymul(2)
            emit_gram(0, 6)
            emit_gram(0, 7)
            emit_gram(1, 0)
            emit_gram(1, 1)
            emit_ymul(3)
            for r in range(2, 8):
                emit_gram(1, r)
            for r in range(8):
                emit_gram(2, r)
            emit_gram(3, 0)
            emit_products_muls()
            emit_gram(3, 1)
            emit_products_psum_a()
            emit_gram(3, 2)
            emit_gram(3, 3)
            emit_products_psum_b()
            emit_gram(3, 4)
            emit_gram(3, 5)
            exclT, posT, sim12T, s12sumT = emit_products_finish()
            emit_gram(3, 6)
            emit_gram(3, 7)

            # ------------- finals ([128, x] layout) ---------------
            rs8T = finp.tile([128, 8], f32, tag="rs8T")
            rs8Tb = finp.tile([128, 8], f32, tag="rs8Tb")
            for r in range(8):
                nc.vector.tensor_reduce(
                    rs8T[:, r : r + 1],
                    rs_acc[:, r * 4 : (r + 1) * 4],
                    axis=AX.X,
                    op=ALU.add,
                )
                nc.vector.tensor_reduce(
                    rs8Tb[:, r : r + 1],
                    rs_accD[:, r * 4 : (r + 1) * 4],
                    axis=AX.X,
                    op=ALU.add,
                )
            nc.vector.tensor_add(rs8T[:], rs8T[:], rs8Tb[:])
            denomT = finp.tile([128, 8], f32, tag="denomT")
            nc.vector.tensor_add(denomT[:], rs8T[:], posT[:])
            nc.vector.tensor_sub(denomT[:], denomT[:], exclT)
            lnden = finp.tile([128, 8], f32, tag="lnden")
            total = finp.tile([128, 1], f32, tag="total")
            nc.scalar.activation(lnden[:], denomT[:], AF.Ln, accum_out=total[:])
            diff = finp.tile([128, 1], bf16, tag="diff")
            nc.vector.tensor_sub(diff[:], total[:], s12sumT[:])
            fin_ps = gpp.tile([128, PW], f32, tag="gp", name="fin_ps")
            nc.tensor.matmul(fin_ps[0:1, 0:1], ones1, diff[:], start=True, stop=True)
            fin = finp.tile([1, 1], f32, tag="fin")
            nc.vector.tensor_copy(fin[:], fin_ps[0:1, 0:1])
            nc.sync.dma_start(out[:], fin[:])

    _split_excess_waits(nc, mybir, max_waits=1)
    return nc


def _split_excess_waits(nc, mybir, max_waits=1):
    """Hoist all but the last sync-wait of every instruction onto same-engine
    NOPs inserted immediately before it (walrus rejects >1 wait/instr)."""
    nsplit = 0
    for f in nc.m.functions:
        for bb in f.blocks:
            new_list = []
            changed = False
            for inst in bb.instructions:
                si = inst.sync_info
                if si is not None and si.on_wait and len(si.on_wait) > max_waits:
                    waits = list(si.on_wait)
                    extra, keep = waits[:-max_waits], waits[-max_waits:]
                    for w in extra:
                        nsplit += 1
                        nop = mybir.InstNoOp(
                            name=f"{inst.name}-wsplit{nsplit}", ins=[], outs=[]
                        )
                        nop.engine = inst.engine
                        nop.sync_info = mybir.SyncInfo(on_wait=[w], on_update=[])
                        nc.register_instruction(nop, overwrite=True)
                        new_list.append(nop)
                    si.on_wait = keep
                    changed = True
                new_list.append(inst)
            if changed:
                if hasattr(bb, "set_instructions"):
                    bb.set_instructions(new_list)
                else:
                    try:
                        bb.instructions[:] = new_list
                    except TypeError:
                        bb.instructions = new_list
    return nsplit


_CACHED_NC = None


def _get_nc():
    global _CACHED_NC
    if _CACHED_NC is None:
        _CACHED_NC = build_program()
    return _CACHED_NC


def make_in_maps(input11: np.ndarray, input22: np.ndarray):
    x1 = np.ascontiguousarray(np.asarray(input11), dtype=np.float32)
    x2 = np.ascontiguousarray(np.asarray(input22), dtype=np.float32)
    x1t = np.ascontiguousarray(x1.T).astype(BF16)  # [D, B]
    x2t = np.ascontiguousarray(x2.T).astype(BF16)  # [D, B]
    in_maps = []
    for i in range(NCORES):
        r0 = i * BLK
        x1tbv = np.empty((D, BW), dtype=BF16)
        x1tbv[:, 0:BLK] = x1t[:, r0 : r0 + BLK]
        x1tbv[:, BLK] = x1t[:, (r0 - 1) % B]
        x2tb = np.ascontiguousarray(x2t[:, r0 : r0 + BLK])
        in_maps.append({"x1t": x1t, "x1tb": x1tbv, "x2t": x2tb})
    return in_maps


def kernel(input11: np.ndarray, input22: np.ndarray, _trace: bool = False):
    from concourse.bass_utils import run_bass_kernel_spmd

    nc = _get_nc()
    in_maps = make_in_maps(input11, input22)
    res = run_bass_kernel_spmd(nc, in_maps, core_ids=list(range(NCORES)), trace=_trace)
    partials = np.array(
        [res.results[i]["out"][0, 0] for i in range(NCORES)], dtype=np.float64
    )
    loss = np.float32(partials.sum() / B)
    if _trace:
        kernel.last_exec_time_ns = res.exec_time_ns
    return loss


kernel.last_exec_time_ns = None
